# revision 1
# baseline (speedup 1.0000x reference)
"""GAT (single-head GATConv + Linear) on 8 Trainium2 NeuronCores.

v4 "H-row" design (dst-node sharding, graph/data parallel):
  - Phase A (fp16): per 128-node tile, one PE matmul of the pre-transposed
    x tile [128ch, 128nodes] with W_aug = [W | W@att_src | W@att_dst | 0pad]
    -> PSUM [128 nodes, 128] f32.  Rows are packed as 256B fp16 records
    [h fp16(64) | a_src f32 | a_dst f32 | pad] (the attention scalars are
    stored exactly via a 4-byte bitcast into fp16 columns 64:68) and written
    to a DRAM table.  Local tiles also stay resident in SBUF (self-loops).
  - Phase B: per dst-window, incoming edges' rows are fetched with two
    non-transposed dma_gathers (256B/row, int16 indices into two overlapping
    32768-row table slices).  The gathered grid [128 lanes, R, 128] directly
    holds h and the attention scalars.
  - dma_gather descriptor generation (~8.5ns/row, GpSimd-serialized) is the
    critical path.  Each gather's num_idxs is statically shrunk to the
    max-over-cores trailing-trimmed slot count (rounded to 16), with window
    lanes ordered by descending degree so pads concentrate at the tail; pad
    slots gather row 0 and are neutralized by a per-core e-mask (-12000
    added to the post-leaky logit).  GpSimd descriptor writes contend with
    the DVE SBUF port, so the per-window math leans on the Scalar engine
    (Lrelu / Exp / Relu-with-scale activations) and fp16 multiplies.
  - Reference quirk reproduced: jax.ops.segment_max on this stack computes a
    segment SUM; we compute w = exp(e - sum_seg e), den = sum w + 1e-16.
"""
import os
import sys

import numpy as np

if "/opt/trn_rl_repo" not in sys.path:
    sys.path.insert(0, "/opt/trn_rl_repo")

import dataclasses

import concourse.bacc as bacc
import concourse.tile as tile
from concourse import mybir
from concourse.bass_utils import run_bass_kernel_spmd
from concourse.masks import make_identity

N = 50000
IN_C, HID, OUT_C = 128, 64, 32
E = 800000
NEG_SLOPE = 0.2
P = 128
NCORES = 8

LOCAL_T = 49                    # windows (dst tiles) per core
LOCAL_ROWS = LOCAL_T * P        # 6272
N_LOCAL_REAL = N // NCORES      # 6250
N_FILL = LOCAL_ROWS - N_LOCAL_REAL  # 22 filler lanes (window 0)
TOTAL_T = 391                   # table tiles per core
TABLE_ROWS = TOTAL_T * P        # 50048
NL_REAL = N - N_LOCAL_REAL      # 43750 non-local real rows
SLICE1_OFF = TABLE_ROWS - 32768  # 17280
LO_NL_CUT = 32768 - LOCAL_ROWS   # non-local positions < this are "lo"
G_CUT = 30281                    # global sort-key prefix approximating the cut
EMASK_PAD = -12000.0             # post-leaky e-mask for pad slots
A_GRP = 16                       # phase-A tiles per DMA batch

f32 = mybir.dt.float32
f16 = mybir.dt.float16
LAST_RESULT = None  # BassKernelResults of the most recent kernel() call

# windows processed big-R first; the first 3 get full-round gathers so the
# freshly-allocated grid buffers are fully overwritten (no stale NaN fp16)
W_ORDER = list(range(LOCAL_T - 1, -1, -1))


# --------------------------------------------------------------------------
# host-side layout
# --------------------------------------------------------------------------

def _lo_class(r):
    return 29 if r > 18 else (18 if r > 12 else 12)


def _hi_class(r):
    return 18 if r > 9 else 9


def _build_layout(src, dst):
    """Per-core node permutations, per-window gather indices, and e-masks."""
    deg = np.bincount(dst, minlength=N).astype(np.int64)  # self-loops excluded

    # approximate (lo,hi) keys from a degree-ordered table prefix so all
    # cores' windows land on aligned degree strata
    order0 = np.argsort(deg, kind="stable")
    inG = np.zeros(N, bool)
    inG[order0[:G_CUT]] = True
    lo_key = np.bincount(dst[inG[src]], minlength=N).astype(np.int64)
    hi_key = deg - lo_key
    order1 = np.lexsort((hi_key, lo_key))    # node ids by (lo_key, hi_key)

    cores = []
    for c in range(NCORES):
        local_nodes = order1[c::NCORES]             # 6250
        is_local = np.zeros(N, bool)
        is_local[local_nodes] = True
        nl_nodes = order1[~is_local[order1]]        # 43750 in key order
        nl_pos = np.full(N, -1, np.int64)
        nl_pos[nl_nodes] = np.arange(nl_nodes.size)

        emask = is_local[dst]
        es, ed = src[emask], dst[emask]
        e_lo = nl_pos[es] < LO_NL_CUT               # local srcs -> lo

        li = np.full(N, -1, np.int64)
        li[local_nodes] = np.arange(local_nodes.size)
        lo_deg = np.bincount(li[ed], minlength=N_LOCAL_REAL,
                             weights=e_lo.astype(np.float64)).astype(np.int64)
        all_deg = np.bincount(li[ed], minlength=N_LOCAL_REAL)
        hi_deg = all_deg - lo_deg

        # window assignment by (lo,hi) ascending; within each window lanes
        # are ordered (lo desc, hi desc) so per-round pads are suffixes
        key = lo_deg * (hi_deg.max() + 2) + hi_deg
        ord_l = np.argsort(key, kind="stable")
        local_sorted = np.empty(LOCAL_ROWS, np.int64)
        lo_arr = np.zeros(LOCAL_ROWS, np.int64)
        hi_arr = np.zeros(LOCAL_ROWS, np.int64)
        tmp_nodes = np.concatenate([np.full(N_FILL, -1, np.int64),
                                    local_nodes[ord_l]])
        tmp_lo = np.concatenate([np.zeros(N_FILL, np.int64), lo_deg[ord_l]])
        tmp_hi = np.concatenate([np.zeros(N_FILL, np.int64), hi_deg[ord_l]])
        for w in range(LOCAL_T):
            sl = slice(w * P, (w + 1) * P)
            local_sorted[sl] = tmp_nodes[sl][::-1]
            lo_arr[sl] = tmp_lo[sl][::-1]
            hi_arr[sl] = tmp_hi[sl][::-1]

        rho = np.full(N, -1, np.int64)
        real_lane = local_sorted >= 0
        rho[local_sorted[real_lane]] = np.flatnonzero(real_lane)
        rho[nl_nodes] = LOCAL_ROWS + np.arange(nl_nodes.size)

        cores.append(dict(
            local_sorted=local_sorted, rho=rho,
            es=es, ed=ed, e_lo=e_lo,
            lo_arr=lo_arr, hi_arr=hi_arr,
        ))

    # per-core slot grids; each gather's static size = max-over-cores
    # trailing-trimmed count, rounded up to 16 idxs
    grids = []
    for cc in cores:
        rho = cc["rho"]
        rd = rho[cc["ed"]]
        w2 = rd // P
        p2 = rd % P
        rho_s = rho[cc["es"]]
        lo2 = cc["e_lo"]
        sk = rd * 2 + (~lo2)
        so = np.argsort(sk, kind="stable")
        sk_s = sk[so]
        grp_start = np.r_[0, np.flatnonzero(np.diff(sk_s)) + 1]
        grp_sizes = np.r_[np.diff(grp_start), sk_s.size - grp_start[-1]]
        r_s = np.arange(sk_s.size) - np.repeat(grp_start, grp_sizes)
        r2 = np.empty(sk_s.size, np.int64)
        r2[so] = r_s
        g = {}
        for w in range(LOCAL_T):
            for kind in (0, 1):
                R = int((cc["lo_arr"] if kind == 0 else cc["hi_arr"])
                        [w * P:(w + 1) * P].max())
                m = (lo2 == (kind == 0)) & (w2 == w)
                jj = r2[m] * P + p2[m]
                vals = rho_s[m] if kind == 0 else rho_s[m] - SLICE1_OFF
                arr = np.zeros(max(R, 0) * P, np.int64)
                val = np.zeros(max(R, 0) * P, bool)
                arr[jj] = vals
                val[jj] = True
                g[(w, kind)] = (arr, val)
        grids.append(g)

    # each gather is sized to the max-over-cores trailing-trimmed count,
    # rounded UP TO FULL 128-ROUNDS: every slot the compute reads is then
    # freshly gathered (pad slots read row 0), so no window ever reads
    # stale SBUF - leftover bytes from earlier NEFFs can be fp16/f32
    # NaN patterns that would poison the masked reductions
    n16 = np.zeros((LOCAL_T, 2), np.int64)
    for w in range(LOCAL_T):
        for kind in (0, 1):
            cnt = 0
            for g in grids:
                val = g[(w, kind)][1]
                nz = np.flatnonzero(val)
                cnt = max(cnt, (int(nz[-1]) + 1) if nz.size else 0)
            n16[w, kind] = (cnt + P - 1) // P * P

    RG = (n16 + P - 1) // P                         # rounds read per gather

    col_off = np.zeros((LOCAL_T, 2), np.int64)
    off = 0
    for w in range(LOCAL_T):
        for kind in (0, 1):
            col_off[w, kind] = off
            off += int(n16[w, kind]) // 16
    S_TOTAL = int(off)

    m_off = np.zeros(LOCAL_T, np.int64)
    off = 0
    for w in range(LOCAL_T):
        m_off[w] = off
        off += 1 + int(RG[w, 0]) + int(RG[w, 1])
    SW = int(off)

    for cc, g in zip(cores, grids):
        idx16 = np.zeros((16, S_TOTAL), np.int16)
        emask_t = np.zeros((P, SW), np.float32)
        for w in range(LOCAL_T):
            for kind in (0, 1):
                nn = int(n16[w, kind])
                if nn == 0:
                    continue
                arr, val = g[(w, kind)]
                a2 = np.zeros(nn, np.int64)
                k = min(nn, arr.size)
                a2[:k] = arr[:k]
                coff = int(col_off[w, kind])
                idx16[:, coff:coff + nn // 16] = \
                    a2.reshape(-1, 16).T.astype(np.int16)
                # e-mask over the FULL read grid (RG rounds): pads + stale
                Rg = int(RG[w, kind])
                vm = np.zeros(Rg * P, bool)
                vm[:k] = val[:k]
                vm = vm.reshape(Rg, P).T             # [P, Rg]
                base = int(m_off[w]) + 1 + (int(RG[w, 0]) if kind else 0)
                emask_t[:, base:base + Rg] = np.where(vm, 0.0, EMASK_PAD)
        cc["idx"] = np.tile(idx16, (8, 1))          # replicate across Q7 cores
        cc["emask"] = emask_t

    return cores, n16, RG, col_off, S_TOTAL, m_off, SW


def _bcast(ap, shape):
    """Free-dim broadcast view: [P,1]-ish AP -> given free shape via 0-steps."""
    new = [ap.ap[0]] + [[0, s] for s in shape]
    return dataclasses.replace(ap, ap=new)


def _build_nc(n16, RG, col_off, S_TOTAL, m_off, SW, skip_bconv):
    nc = bacc.Bacc(None, target_bir_lowering=False, num_devices=NCORES,
                   num_swdge_queues=4)

    xtT_in = nc.dram_tensor("xtT_in", [TABLE_ROWS, IN_C], f16, kind="ExternalInput")
    idx_in = nc.dram_tensor("idx_in", [P, S_TOTAL], mybir.dt.int16, kind="ExternalInput")
    emask_in = nc.dram_tensor("emask_in", [P, SW], f32, kind="ExternalInput")
    w_in = nc.dram_tensor("w_in", [IN_C, P], f16, kind="ExternalInput")
    wlin_in = nc.dram_tensor("wlin_in", [P, OUT_C], f32, kind="ExternalInput")
    blin_in = nc.dram_tensor("blin_in", [P, OUT_C], f32, kind="ExternalInput")
    bconv_in = nc.dram_tensor("bconv_in", [P, HID], f32, kind="ExternalInput")
    y_out = nc.dram_tensor("y_out", [LOCAL_ROWS, OUT_C], f32, kind="ExternalOutput")
    h_dram = nc.dram_tensor("h_scratch", [TABLE_ROWS, P], f16)

    lo_class, hi_class = _lo_class, _hi_class
    W1_all = {w: 1 + int(RG[w, 0]) + int(RG[w, 1]) for w in range(LOCAL_T)}

    with tile.TileContext(nc) as tc:
        with (
            tc.tile_pool(name="const", bufs=1) as cpool,
            tc.tile_pool(name="pa", bufs=3) as pa,
            tc.tile_pool(name="pah", bufs=3) as pah,
            tc.tile_pool(name="psa", bufs=4, space="PSUM") as psa,
            tc.tile_pool(name="pg", bufs=1) as pg,
            tc.tile_pool(name="pb", bufs=3) as pb,
            tc.tile_pool(name="pm", bufs=2) as pm,
            tc.tile_pool(name="psb", bufs=2, space="PSUM") as psb,
        ):
            w_sb = cpool.tile([IN_C, P], f16)
            nc.sync.dma_start(w_sb[:], w_in[:])
            wlin_sb = cpool.tile([P, OUT_C], f32)
            nc.sync.dma_start(wlin_sb[:], wlin_in[:])
            blin_sb = cpool.tile([P, OUT_C], f32)
            nc.sync.dma_start(blin_sb[:], blin_in[:])
            bconv_sb = cpool.tile([P, HID], f32)
            nc.sync.dma_start(bconv_sb[:], bconv_in[:])
            idx_sb = cpool.tile([P, S_TOTAL], mybir.dt.int16)
            nc.sync.dma_start(idx_sb[:], idx_in[:])
            emask_sb = cpool.tile([P, SW], f32)
            nc.sync.dma_start(emask_sb[:], emask_in[:])
            ident = cpool.tile([P, P], f32)
            make_identity(nc, ident[:])
            hself_all = cpool.tile([P, LOCAL_T, P], f16)
            # persistent rotating owT buffers; partition 64 = 1.0 feeds the
            # b_lin row folded into wlin, partitions 65:128 stay zero
            owT_buf = cpool.tile([P, 3, P], f32)
            nc.gpsimd.memset(owT_buf[HID:P, :, :], 0.0)
            nc.gpsimd.memset(owT_buf[HID:HID + 1, :, :], 1.0)

            # ---------------- phase A: h-row table ----------------
            t = 0
            while t < TOTAL_T:
                k = min(A_GRP, TOTAL_T - t)
                xt8 = pa.tile([P, k, P], f16, tag="xt")
                src_view = xtT_in[t * P:(t + k) * P, :].rearrange(
                    "(g c) n -> c g n", c=P)
                nc.sync.dma_start(xt8[:], src_view)
                row8 = pah.tile([P, A_GRP, P], f16, tag="row8")
                for j in range(k):
                    ps = psa.tile([P, P], f32, space="PSUM", tag="psA")
                    nc.tensor.matmul(ps[:], xt8[:, j, :], w_sb[:],
                                     start=True, stop=True)
                    # split the big copies across ACT and DVE; the f32
                    # bitcast write stays on DVE (v3-proven path)
                    if (t + j) % 2 == 0:
                        nc.scalar.copy(row8[:, j, :], ps[:])
                    else:
                        nc.vector.tensor_copy(row8[:, j, :], ps[:])
                    nc.vector.tensor_copy(
                        row8[:, j, HID:HID + 4].bitcast(f32),
                        ps[:, HID:HID + 2])
                    if t + j < LOCAL_T:
                        nc.scalar.copy(hself_all[:, t + j, :], row8[:, j, :])
                dst_view = h_dram[t * P:(t + k) * P, :].rearrange(
                    "(g p) c -> p g c", p=P)
                nc.sync.dma_start(dst_view, row8[:, 0:k, :])
                t += k

            slice0 = h_dram[0:32768, :]
            slice1 = h_dram[SLICE1_OFF:TABLE_ROWS, :]

            # ---------------- phase B: per-window attention ----------------
            for wi, w in enumerate(W_ORDER):
                RL, RH = int(RG[w, 0]), int(RG[w, 1])
                nlo, nhi = int(n16[w, 0]), int(n16[w, 1])
                W1 = W1_all[w]
                Hlo = Hhi = None
                if nlo:
                    cl = lo_class(RL)
                    Hlo = pg.tile([P, cl, P], f16, tag=f"Hlo{cl}",
                                  bufs=2 if cl == 29 else (4 if cl == 18 else 5))
                    nc.gpsimd.dma_gather(
                        out_ap=Hlo[:, 0:RL, :], in_ap=slice0,
                        idxs_ap=idx_sb[:, int(col_off[w, 0]):int(col_off[w, 0]) + nlo // 16],
                        num_idxs=nlo, num_idxs_reg=nlo, elem_size=P,
                        single_packet=False, queue_num=(2 * wi) % 4)
                if nhi:
                    cl = hi_class(RH)
                    Hhi = pg.tile([P, cl, P], f16, tag=f"Hhi{cl}",
                                  bufs=4 if cl == 18 else 5)
                    nc.gpsimd.dma_gather(
                        out_ap=Hhi[:, 0:RH, :], in_ap=slice1,
                        idxs_ap=idx_sb[:, int(col_off[w, 1]):int(col_off[w, 1]) + nhi // 16],
                        num_idxs=nhi, num_idxs_reg=nhi, elem_size=P,
                        single_packet=False, queue_num=(2 * wi + 1) % 4)
                h_self = hself_all[:, w, :]

                # e columns: [self | lo rounds | hi rounds], leaky via ACT
                a_self = h_self[:, HID:HID + 4].bitcast(f32)   # [P, 2]
                adst = a_self[:, 1:2]
                e_sb = pb.tile([P, W1], f32, tag="e")
                nc.scalar.activation(e_sb[:, 0:1], a_self[:, 0:1],
                                     mybir.ActivationFunctionType.Identity,
                                     bias=adst)
                for (Ht, R, o) in ((Hlo, RL, 1), (Hhi, RH, 1 + RL)):
                    if R == 0:
                        continue
                    a_src = Ht[:, 0:R, HID:HID + 4].bitcast(f32)  # [P, R, 2]
                    nc.scalar.activation(e_sb[:, o:o + R], a_src[:, :, 0],
                                         mybir.ActivationFunctionType.Identity,
                                         bias=adst)
                t_sb = pb.tile([P, W1], f32, tag="t")
                nc.vector.tensor_scalar_mul(t_sb[:], e_sb[:], NEG_SLOPE)
                nc.vector.tensor_tensor(out=e_sb[:], in0=e_sb[:], in1=t_sb[:],
                                        op=mybir.AluOpType.max)
                nc.vector.tensor_tensor(
                    out=e_sb[:], in0=e_sb[:],
                    in1=emask_sb[:, int(m_off[w]):int(m_off[w]) + W1],
                    op=mybir.AluOpType.add)
                # reference's "segment_max" is a segment SUM on this stack:
                # m = sum of e over real slots; w = exp(e - m)
                mask = pb.tile([P, W1], f32, tag="mask")
                nc.vector.tensor_scalar(
                    mask[:], e_sb[:], -1.0e4, -1.0,
                    op0=mybir.AluOpType.is_gt, op1=mybir.AluOpType.mult)
                mneg = pb.tile([P, 1], f32, tag="mneg")
                nc.vector.tensor_tensor(out=t_sb[:], in0=e_sb[:], in1=mask[:],
                                        op=mybir.AluOpType.mult)
                nc.vector.tensor_reduce(mneg[:], t_sb[:], axis=mybir.AxisListType.X,
                                        op=mybir.AluOpType.add)
                wgt = pb.tile([P, W1], f32, tag="w")
                den = pb.tile([P, 1], f32, tag="den")
                nc.scalar.activation(wgt[:], e_sb[:], mybir.ActivationFunctionType.Exp,
                                     bias=mneg[:, 0:1], accum_out=den[:, 0:1])
                rec = pb.tile([P, 1], f32, tag="rec")
                nc.vector.tensor_scalar_add(rec[:], den[:], 1e-16)
                nc.vector.reciprocal(rec[:], rec[:])
                # normalize BEFORE the fp16 cast: alpha in [0,1] cannot
                # overflow fp16, unlike exp(e - m) when m is very negative
                wgt16 = pb.tile([P, W1], f16, tag="w16")
                nc.scalar.activation(wgt16[:], wgt[:],
                                     mybir.ActivationFunctionType.Identity,
                                     scale=rec[:, 0:1])

                # ---------------- weighted message sum (fp16) ----------------
                msgsT = pm.tile([P, HID, W1], f16, tag="msgsT")
                Hs = dataclasses.replace(
                    h_self[:, 0:HID], ap=[h_self[:, 0:HID].ap[0], [1, HID], [1, 1]])
                ws = dataclasses.replace(
                    wgt16[:, 0:1], ap=[wgt16[:].ap[0], [0, HID], [1, 1]])
                nc.vector.tensor_tensor(out=msgsT[:, :, 0:1], in0=Hs, in1=ws,
                                        op=mybir.AluOpType.mult)
                for (Ht, R, o) in ((Hlo, RL, 1), (Hhi, RH, 1 + RL)):
                    if R == 0:
                        continue
                    HvT = dataclasses.replace(
                        Ht[:, 0:R, 0:HID], ap=[Ht[:].ap[0], [1, HID], [P, R]])
                    w_b = dataclasses.replace(
                        wgt16[:, o:o + R], ap=[wgt16[:].ap[0], [0, HID], [1, R]])
                    nc.vector.tensor_tensor(out=msgsT[:, :, o:o + R], in0=HvT,
                                            in1=w_b, op=mybir.AluOpType.mult)
                num = pb.tile([P, HID], f32, tag="num")
                nc.vector.tensor_reduce(num[:], msgsT[:], axis=mybir.AxisListType.X,
                                        op=mybir.AluOpType.add)

                ow = pb.tile([P, HID], f32, tag="ow")
                nc.vector.tensor_scalar_max(ow[:], num[:], 0.0)
                if not skip_bconv:
                    nc.vector.tensor_tensor(out=ow[:], in0=ow[:], in1=bconv_sb[:],
                                            op=mybir.AluOpType.add)
                    nc.vector.tensor_scalar_max(ow[:], ow[:], 0.0)

                owT_ps = psb.tile([HID, P], f32, space="PSUM", tag="owT")
                nc.tensor.transpose(owT_ps[:], ow[:], ident[:])
                # K=64 matmuls alternating with PE transposes crash the device;
                # pad lhsT to K=128 (partitions 64:128 of owT_buf stay zero)
                owT = owT_buf[:, wi % 3, :]
                nc.scalar.copy(owT_buf[0:HID, wi % 3, :], owT_ps[:])
                y_ps = psb.tile([P, OUT_C], f32, space="PSUM", tag="y")
                nc.tensor.matmul(y_ps[:], owT, wlin_sb[:], start=True, stop=True)
                y_sb = pb.tile([P, OUT_C], f32, tag="ysb")
                nc.scalar.copy(y_sb[:], y_ps[:])
                nc.sync.dma_start(y_out[w * P:(w + 1) * P, :], y_sb[:])

    nc.compile()
    return nc


def kernel(x, edge_index, W, att_src, att_dst, bias_conv, W_lin, b_lin):
    global LAST_RESULT
    x = np.asarray(x, np.float32)
    edge_index = np.asarray(edge_index)
    W = np.asarray(W, np.float32)
    att_src = np.asarray(att_src, np.float32)
    att_dst = np.asarray(att_dst, np.float32)
    bias_conv = np.asarray(bias_conv, np.float32)
    W_lin = np.asarray(W_lin, np.float32)
    b_lin = np.asarray(b_lin, np.float32)
    src = np.asarray(edge_index[0], np.int64)
    dst = np.asarray(edge_index[1], np.int64)

    cores, n16, RG, col_off, S_TOTAL, m_off, SW = _build_layout(src, dst)

    W_aug = np.concatenate(
        [W, (W @ att_src)[:, None], (W @ att_dst)[:, None],
         np.zeros((IN_C, P - HID - 2), np.float32)], axis=1
    ).astype(np.float16)
    blin_b = np.tile(b_lin[None, :], (P, 1)).astype(np.float32)
    bconv_b = np.tile(bias_conv[None, :], (P, 1)).astype(np.float32)
    wlin_pad = np.vstack([W_lin, b_lin[None, :],
                          np.zeros((P - HID - 1, OUT_C), np.float32)])
    skip_bconv = bool(np.all(bias_conv == 0.0))

    nc = _build_nc(n16, RG, col_off, S_TOTAL, m_off, SW, skip_bconv)

    x16 = x.astype(np.float16)
    in_maps = []
    for cc in cores:
        rows = np.full(TABLE_ROWS, -1, np.int64)
        rows[0:LOCAL_ROWS] = cc["local_sorted"]
        nl = np.flatnonzero(cc["rho"] >= LOCAL_ROWS)
        rows[LOCAL_ROWS:LOCAL_ROWS + NL_REAL] = nl[np.argsort(cc["rho"][nl])]
        xt = np.zeros((TABLE_ROWS, IN_C), np.float16)
        real = rows >= 0
        xt[real] = x16[rows[real]]
        # per-tile transpose: phase A loads [128ch, 128nodes] blocks
        xtT = np.ascontiguousarray(
            xt.reshape(TOTAL_T, P, IN_C).transpose(0, 2, 1)
        ).reshape(TABLE_ROWS, IN_C)
        in_maps.append({
            "xtT_in": xtT, "idx_in": cc["idx"],
            "emask_in": cc["emask"], "w_in": W_aug,
            "wlin_in": wlin_pad, "blin_in": blin_b, "bconv_in": bconv_b,
        })

    res = run_bass_kernel_spmd(nc, in_maps, core_ids=list(range(NCORES)))
    LAST_RESULT = res

    y = np.empty((N, OUT_C), np.float32)
    for c, cc in enumerate(cores):
        yc = np.asarray(res.results[c]["y_out"])
        ls = cc["local_sorted"]
        real = ls >= 0
        y[ls[real]] = yc[real]
    return y

